# revision 5
# baseline (speedup 1.0000x reference)
"""NCC loss (9x9x9 box normalized cross-correlation) on 8 TRN2 NeuronCores.

Inputs: y_pred, y_true f32 (2,1,128,128,128). Output: scalar f32 loss.

Sharding: D axis (dim 2) split 4-ways per batch -> 8 slabs of 32 D-slices,
each with a 4-slice halo (host zero-pads volume edges).

Per core, fp16 on-chip with f32 PSUM, band taps = 1/9 so every pass emits
window MEANS (scale cancels exactly in cc = cross^2/(Iv*Jv)):
  vols   : I, J, I*I, J*J, I*J              (ACT converts, DVE products)
  pass 1 : per-d-slice flip matmul vs 9-band bh -> H box   [W, (h', d)]
  pass 2 : per-group flip matmul vs bh          -> W box   [(l,d), (g, w)]
  pass 3 : block-band b3 [120,96]               -> D box, f32 PSUM
  ptw    : cc per voxel; per-partition sums via tensor_tensor_reduce
Host: sum per-core partials, loss = -sum / N.

Group scheme: 43 groups of 3 h'-slices; groups 0..41 cover h' 0..125,
group 42 covers h' 126..127 (band rows limited to 80/64).
"""

import numpy as np

import concourse.bacc as bacc
import concourse.tile as tile
from concourse import mybir
from concourse.bass_utils import run_bass_kernel_spmd

F32 = mybir.dt.float32
FP16 = mybir.dt.float16
ALU = mybir.AluOpType
ACTF = mybir.ActivationFunctionType

B, D, H, W = 2, 128, 128, 128
DL, PAD = 32, 4
DH = DL + 2 * PAD            # 40
NG = 43
TAP = 1.0 / 9.0
N_TOT = float(B * D * H * W)

_CACHE = {}


def _build():
    nc = bacc.Bacc(trn_type="TRN2", target_bir_lowering=False)

    i_dram = nc.dram_tensor("i_slab", [DH, H, W], F32, kind="ExternalInput")
    j_dram = nc.dram_tensor("j_slab", [DH, H, W], F32, kind="ExternalInput")
    out_dram = nc.dram_tensor("partials", [128, 1], F32, kind="ExternalOutput")

    with tile.TileContext(nc) as tc:
        with (
            tc.tile_pool(name="bands", bufs=1) as bands,
            tc.tile_pool(name="t2", bufs=1) as t2p,
            tc.tile_pool(name="accp", bufs=1) as accp,
        ):
            # ---------- band matrices (taps 1/9) ----------
            # bh[p, j] = 1/9 iff |p - j| <= 4
            bh = bands.tile([128, 128], FP16)
            nc.gpsimd.memset(bh[:, :], TAP)
            nc.gpsimd.affine_select(bh[:, :], bh[:, :], pattern=[[-1, 128]],
                                    compare_op=ALU.is_ge, fill=0.0,
                                    base=PAD, channel_multiplier=1)
            nc.gpsimd.affine_select(bh[:, :], bh[:, :], pattern=[[1, 128]],
                                    compare_op=ALU.is_ge, fill=0.0,
                                    base=PAD, channel_multiplier=-1)
            # b3[p, (l, j)] = 1/9 iff 0 <= p - 40l - j <= 8, rows 120+ zero
            b3 = bands.tile([128, 3, 32], FP16)
            nc.gpsimd.memset(b3[:, :, :], TAP)
            nc.gpsimd.affine_select(b3[:, :, :], b3[:, :, :],
                                    pattern=[[-40, 3], [-1, 32]],
                                    compare_op=ALU.is_ge, fill=0.0,
                                    base=0, channel_multiplier=1)
            nc.gpsimd.affine_select(b3[:, :, :], b3[:, :, :],
                                    pattern=[[40, 3], [1, 32]],
                                    compare_op=ALU.is_ge, fill=0.0,
                                    base=8, channel_multiplier=-1)
            nc.gpsimd.affine_select(b3[:, :, :], b3[:, :, :],
                                    pattern=[[0, 3], [0, 32]],
                                    compare_op=ALU.is_ge, fill=0.0,
                                    base=119, channel_multiplier=-1)
            b3f = b3.rearrange("p l j -> p (l j)")

            acc_all = accp.tile([128, 12], F32)
            nc.vector.memset(acc_all[:, :], 0.0)

            # ---------- load + stage A: the five fp16 volumes ----------
            cm_vol = tc.tile_pool(name="vols", bufs=1)
            volsp = cm_vol.__enter__()
            cm_in = tc.tile_pool(name="inputs", bufs=1)
            inputs = cm_in.__enter__()

            i_f32 = inputs.tile([128, DH, W], F32)
            j_f32 = inputs.tile([128, DH, W], F32)
            i_re = i_dram.rearrange("d h w -> h d w")
            j_re = j_dram.rearrange("d h w -> h d w")
            for q in range(4):
                s = slice(q * 10, q * 10 + 10)
                nc.sync.dma_start(out=i_f32[:, s, :], in_=i_re[:, s, :])
                nc.sync.dma_start(out=j_f32[:, s, :], in_=j_re[:, s, :])

            vols = [volsp.tile([128, DH, W], FP16, tag=f"vol{v}",
                               name=f"vol{v}") for v in range(5)]
            vI, vJ, vI2, vJ2, vIJ = vols
            for q in range(4):
                s = slice(q * 10, q * 10 + 10)
                nc.scalar.copy(vI[:, s, :], i_f32[:, s, :])
                nc.scalar.copy(vJ[:, s, :], j_f32[:, s, :])
                nc.vector.tensor_tensor(out=vI2[:, s, :], in0=vI[:, s, :],
                                        in1=vI[:, s, :], op=ALU.mult)
                nc.vector.tensor_tensor(out=vJ2[:, s, :], in0=vJ[:, s, :],
                                        in1=vJ[:, s, :], op=ALU.mult)
                nc.vector.tensor_tensor(out=vIJ[:, s, :], in0=vI[:, s, :],
                                        in1=vJ[:, s, :], op=ALU.mult)
            cm_in.__exit__(None, None, None)

            # ---------- passes 1+2, vol-pipelined ----------
            cm_t1 = tc.tile_pool(name="t1", bufs=2)
            t1p = cm_t1.__enter__()
            cm_ps1 = tc.tile_pool(name="ps1", bufs=2, space="PSUM")
            ps1p = cm_ps1.__enter__()
            cm_ps2 = tc.tile_pool(name="ps2", bufs=2, space="PSUM")
            ps2p = cm_ps2.__enter__()

            t2 = [t2p.tile([128, NG, 128], FP16, tag=f"t2_{v}", name=f"t2_{v}")
                  for v in range(5)]

            rr = [0]

            def copy_rr(dst, src):
                # GPSIMD cannot touch PSUM: alternate ACT/DVE only
                k = rr[0] % 2
                rr[0] += 1
                if k == 0:
                    nc.scalar.copy(dst, src)
                else:
                    nc.vector.tensor_copy(dst, src)

            def pass1(v):
                t1v = t1p.tile([128, 128, DH], FP16, tag="t1", name=f"t1_{v}")
                for db in range(5):
                    ps = ps1p.tile([128, 8, 128], F32, tag="ps1")
                    for k in range(8):
                        nc.tensor.matmul(out=ps[:, k, :],
                                         lhsT=vols[v][:, db * 8 + k, :],
                                         rhs=bh[:, :])
                    dd = slice(db * 8, db * 8 + 8)
                    copy_rr(t1v[:, :, dd].rearrange("p h d -> p d h"),
                            ps[:, :, :])
                return t1v

            def pass2(v, t1v):
                for gb in range(6):
                    gs = list(range(gb * 8, min(gb * 8 + 8, NG)))
                    ps = ps2p.tile([128, 8, 128], F32, tag="ps2")
                    for k, g in enumerate(gs):
                        h0, hn = (3 * g, 3) if g < 42 else (126, 2)
                        lhs = t1v[:, h0:h0 + hn, :].rearrange(
                            "p l d -> p (l d)")
                        nc.tensor.matmul(out=ps[0:hn * DH, k, :],
                                         lhsT=lhs,
                                         rhs=bh[:, :])
                    if gb < 5:
                        copy_rr(t2[v][0:120, gs[0]:gs[0] + 8, :],
                                ps[0:120, :, :])
                    else:
                        copy_rr(t2[v][0:120, 40:42, :], ps[0:120, 0:2, :])
                        copy_rr(t2[v][0:80, 42, :], ps[0:80, 2, :])

            prev = None
            for v in range(5):
                t1v = pass1(v)
                if prev is not None:
                    pass2(*prev)
                prev = (v, t1v)
            pass2(*prev)

            cm_ps2.__exit__(None, None, None)
            cm_ps1.__exit__(None, None, None)
            cm_t1.__exit__(None, None, None)
            cm_vol.__exit__(None, None, None)

            # ---------- pass 3 + pointwise ----------
            cm_ps3 = tc.tile_pool(name="ps3", bufs=7, space="PSUM")
            ps3p = cm_ps3.__enter__()
            cm_ptw = tc.tile_pool(name="ptw", bufs=2)
            ptw = cm_ptw.__enter__()

            for ci in range(12):
                if ci < 10:
                    g0, ng, P, F, Kk = ci * 4, 4, 96, 512, 120
                    lhs3 = b3f[0:120, 0:96]
                elif ci == 10:
                    g0, ng, P, F, Kk = 40, 2, 96, 256, 120
                    lhs3 = b3f[0:120, 0:96]
                else:
                    g0, ng, P, F, Kk = 42, 1, 64, 128, 80
                    lhs3 = b3f[0:80, 0:64]

                ps5 = []
                for v in range(5):
                    pt = ps3p.tile([96, 512], F32, tag="ps3")
                    nc.tensor.matmul(
                        out=pt[0:P, 0:F],
                        lhsT=lhs3,
                        rhs=t2[v][0:Kk, g0:g0 + ng, :].rearrange(
                            "p g w -> p (g w)"))
                    ps5.append(pt)
                sA = ps5[0][0:P, 0:F]
                sB = ps5[1][0:P, 0:F]
                sC = ps5[2][0:P, 0:F]
                sD = ps5[3][0:P, 0:F]
                sE = ps5[4][0:P, 0:F]

                def st(tag, dt=FP16):
                    return ptw.tile([96, 512], dt, tag=tag,
                                    name=tag)[0:P, 0:F]

                qA, bA, qB, bC = st("qA"), st("bA"), st("qB"), st("bC")
                nc.scalar.activation(qA, sA, ACTF.Square)
                nc.scalar.copy(bA, sA)
                nc.scalar.activation(qB, sB, ACTF.Square)
                nc.scalar.copy(bC, sC)

                Pm = st("Pm")
                nc.vector.scalar_tensor_tensor(out=Pm, in0=sB, scalar=1.0,
                                               in1=bA, op0=ALU.bypass,
                                               op1=ALU.mult)
                Iv, Jv, cross = st("Iv"), st("Jv"), st("cross")
                nc.gpsimd.tensor_tensor(out=Iv, in0=bC, in1=qA,
                                        op=ALU.subtract)
                nc.vector.scalar_tensor_tensor(out=Jv, in0=sD, scalar=1.0,
                                               in1=qB, op0=ALU.bypass,
                                               op1=ALU.subtract)
                nc.vector.scalar_tensor_tensor(out=cross, in0=sE, scalar=1.0,
                                               in1=Pm, op0=ALU.bypass,
                                               op1=ALU.subtract)
                num = st("num")
                nc.scalar.activation(num, cross, ACTF.Square)
                dene = st("dene", F32)
                nc.gpsimd.tensor_tensor(out=dene, in0=Iv, in1=Jv,
                                        op=ALU.mult)
                rec = st("rec", F32)
                nc.vector.reciprocal_approx_fast(out=rec, in_=dene)
                ccs = st("ccs")
                with nc.allow_low_precision(reason="cc scratch fp16"):
                    nc.vector.tensor_tensor_reduce(
                        out=ccs, in0=num, in1=rec, scale=1.0, scalar=0.0,
                        op0=ALU.mult, op1=ALU.add,
                        accum_out=acc_all[0:P, ci:ci + 1])

            cm_ptw.__exit__(None, None, None)
            cm_ps3.__exit__(None, None, None)

            accs = accp.tile([128, 1], F32)
            nc.vector.tensor_reduce(out=accs[:, :], in_=acc_all[:, :],
                                    axis=mybir.AxisListType.X, op=ALU.add)
            nc.sync.dma_start(out=out_dram[:, :], in_=accs[:, :])

    nc.compile()
    return nc


def kernel(y_pred: np.ndarray, y_true: np.ndarray) -> np.ndarray:
    y_pred = np.ascontiguousarray(np.asarray(y_pred, dtype=np.float32))
    y_true = np.ascontiguousarray(np.asarray(y_true, dtype=np.float32))

    if "nc" not in _CACHE:
        _CACHE["nc"] = _build()
    nc = _CACHE["nc"]

    in_maps = []
    for core in range(8):
        b = core // 4
        d0 = (core % 4) * DL
        islab = np.zeros((DH, H, W), np.float32)
        jslab = np.zeros((DH, H, W), np.float32)
        lo, hi = d0 - PAD, d0 + DL + PAD
        slo, shi = max(lo, 0), min(hi, D)
        islab[slo - lo:shi - lo] = y_true[b, 0, slo:shi]
        jslab[slo - lo:shi - lo] = y_pred[b, 0, slo:shi]
        in_maps.append({"i_slab": islab, "j_slab": jslab})

    res = run_bass_kernel_spmd(nc, in_maps, core_ids=list(range(8)))
    total = 0.0
    for r in res.results:
        total += float(np.asarray(r["partials"], np.float64).sum())
    return np.float32(-total / N_TOT)


if __name__ == "__main__":
    rng = np.random.default_rng(0)
    yp = rng.standard_normal((B, 1, D, H, W), dtype=np.float32)
    yt = rng.standard_normal((B, 1, D, H, W), dtype=np.float32)
    print("loss:", kernel(yp, yt))


# revision 10
# speedup vs baseline: 1.0268x; 1.0268x over previous
"""NCC loss (9x9x9 box normalized cross-correlation) on 8 TRN2 NeuronCores.

Inputs: y_pred, y_true f32 (2,1,128,128,128). Output: scalar f32 loss.

Sharding: D axis (dim 2) split 4-ways per batch -> 8 slabs of 32 D-slices,
each with a 4-slice halo (host zero-pads volume edges).

Per core, fp16 on-chip with f32 PSUM, band taps = 1/9 so every pass emits
window MEANS (the tap scale cancels exactly in cc = cross^2/(Iv*Jv)):
  vols   : I, J, I*I, J*J, I*J                 (ACT converts, DVE products)
  pass 1 : per-d-slice flip matmul vs 9-band bh -> H box   [W, (h', d)]
  pass 2 : per-group flip matmul vs bh          -> W box   [(l,d), (g, w)]
  pass 3 : block-band b3 [120,96]               -> D box, f32 PSUM
  ptw    : cc per voxel, accumulated via STT accum_out
Passes 2+3+pointwise run fused per 4-group block so ACT/DVE/Pool pointwise
overlaps PE matmuls. Host: sum per-core partials, loss = -sum / N.

Group scheme: 43 groups of 3 h'-slices; groups 0..41 cover h' 0..125,
group 42 covers h' 126..127 (band rows limited to 80/64).
"""

import numpy as np

import concourse.bacc as bacc
import concourse.tile as tile
from concourse import mybir
from concourse.bass_utils import run_bass_kernel_spmd

F32 = mybir.dt.float32
FP16 = mybir.dt.float16
ALU = mybir.AluOpType
ACTF = mybir.ActivationFunctionType

B, D, H, W = 2, 1, 128, 128  # D redefined below; keep names local
B, Dv, H, W = 2, 128, 128, 128
DL, PAD = 32, 4
DH = DL + 2 * PAD            # 40
NG = 43
TAP = 1.0 / 9.0
N_TOT = float(B * Dv * H * W)

_CACHE = {}


def _build():
    nc = bacc.Bacc(trn_type="TRN2", target_bir_lowering=False)

    i_dram = nc.dram_tensor("i_slab", [DH, H, W], F32, kind="ExternalInput")
    j_dram = nc.dram_tensor("j_slab", [DH, H, W], F32, kind="ExternalInput")
    out_dram = nc.dram_tensor("partials", [128, 1], F32, kind="ExternalOutput")

    with tile.TileContext(nc) as tc:
        with (
            tc.tile_pool(name="bands", bufs=1) as bands,
            tc.tile_pool(name="t2", bufs=1) as t2p,
            tc.tile_pool(name="accp", bufs=1) as accp,
        ):
            # ---------- band matrices (taps 1/9) ----------
            bh = bands.tile([128, 128], FP16)
            nc.gpsimd.memset(bh[:, :], TAP)
            nc.gpsimd.affine_select(bh[:, :], bh[:, :], pattern=[[-1, 128]],
                                    compare_op=ALU.is_ge, fill=0.0,
                                    base=PAD, channel_multiplier=1)
            nc.gpsimd.affine_select(bh[:, :], bh[:, :], pattern=[[1, 128]],
                                    compare_op=ALU.is_ge, fill=0.0,
                                    base=PAD, channel_multiplier=-1)
            b3 = bands.tile([128, 3, 32], FP16)
            nc.gpsimd.memset(b3[:, :, :], TAP)
            nc.gpsimd.affine_select(b3[:, :, :], b3[:, :, :],
                                    pattern=[[-40, 3], [-1, 32]],
                                    compare_op=ALU.is_ge, fill=0.0,
                                    base=0, channel_multiplier=1)
            nc.gpsimd.affine_select(b3[:, :, :], b3[:, :, :],
                                    pattern=[[40, 3], [1, 32]],
                                    compare_op=ALU.is_ge, fill=0.0,
                                    base=8, channel_multiplier=-1)
            nc.gpsimd.affine_select(b3[:, :, :], b3[:, :, :],
                                    pattern=[[0, 3], [0, 32]],
                                    compare_op=ALU.is_ge, fill=0.0,
                                    base=119, channel_multiplier=-1)
            b3f = b3.rearrange("p l j -> p (l j)")

            acc_all = accp.tile([128, 12], F32)
            nc.vector.memset(acc_all[:, :], 0.0)

            # ---------- t1 for all 5 vols lives through the fused loop ----
            cm_t1 = tc.tile_pool(name="t1", bufs=1)
            t1p = cm_t1.__enter__()
            t1s = [t1p.tile([128, 128, DH], FP16, tag=f"t1_{v}",
                            name=f"t1_{v}") for v in range(5)]

            # ---------- load + stage A ----------
            cm_vol = tc.tile_pool(name="vols", bufs=1)
            volsp = cm_vol.__enter__()
            cm_in = tc.tile_pool(name="inputs", bufs=1)
            inputs = cm_in.__enter__()

            i_f32 = inputs.tile([128, DH, W], F32)
            j_f32 = inputs.tile([128, DH, W], F32)
            i_re = i_dram.rearrange("d h w -> h d w")
            j_re = j_dram.rearrange("d h w -> h d w")
            for q in range(4):
                s = slice(q * 10, q * 10 + 10)
                nc.sync.dma_start(out=i_f32[:, s, :], in_=i_re[:, s, :])
                nc.sync.dma_start(out=j_f32[:, s, :], in_=j_re[:, s, :])

            vols = [volsp.tile([128, DH, W], FP16, tag=f"vol{v}",
                               name=f"vol{v}") for v in range(5)]
            vI, vJ, vI2, vJ2, vIJ = vols
            for q in range(4):
                s = slice(q * 10, q * 10 + 10)
                nc.scalar.copy(vI[:, s, :], i_f32[:, s, :])
                nc.scalar.copy(vJ[:, s, :], j_f32[:, s, :])
                nc.vector.tensor_tensor(out=vI2[:, s, :], in0=vI[:, s, :],
                                        in1=vI[:, s, :], op=ALU.mult)
                nc.vector.tensor_tensor(out=vJ2[:, s, :], in0=vJ[:, s, :],
                                        in1=vJ[:, s, :], op=ALU.mult)
                nc.vector.tensor_tensor(out=vIJ[:, s, :], in0=vI[:, s, :],
                                        in1=vJ[:, s, :], op=ALU.mult)
            cm_in.__exit__(None, None, None)

            # ---------- pass 1, all vols ----------
            cm_ps1 = tc.tile_pool(name="ps1", bufs=3, space="PSUM")
            ps1p = cm_ps1.__enter__()
            rr = [0]

            def copy_rr(dst, src):
                # GPSIMD cannot touch PSUM: alternate ACT/DVE
                k = rr[0] % 2
                rr[0] += 1
                if k == 0:
                    nc.scalar.copy(dst, src)
                else:
                    nc.vector.tensor_copy(dst, src)

            for v in range(5):
                for db in range(5):
                    ps = ps1p.tile([128, 8, 128], F32, tag="ps1")
                    for k in range(8):
                        nc.tensor.matmul(out=ps[:, k, :],
                                         lhsT=vols[v][:, db * 8 + k, :],
                                         rhs=bh[:, :])
                    dd = slice(db * 8, db * 8 + 8)
                    copy_rr(t1s[v][:, :, dd].rearrange("p h d -> p d h"),
                            ps[:, :, :])
            cm_ps1.__exit__(None, None, None)
            cm_vol.__exit__(None, None, None)

            # ---------- fused pass2 + pass3 + pointwise per 4-group block --
            cm_ps2 = tc.tile_pool(name="ps2", bufs=2, space="PSUM")
            ps2p = cm_ps2.__enter__()
            cm_ps3 = tc.tile_pool(name="ps3", bufs=5, space="PSUM")
            ps3p = cm_ps3.__enter__()
            cm_ptw = tc.tile_pool(name="ptw", bufs=2)
            ptw = cm_ptw.__enter__()

            t2 = [t2p.tile([128, NG, 128], FP16, tag=f"t2_{v}",
                           name=f"t2_{v}") for v in range(5)]

            def pass2_block(v, gs):
                """Groups gs (<=4) of vol v -> t2[v]."""
                ps = ps2p.tile([128, 4, 128], F32, tag="ps2")
                for k, g in enumerate(gs):
                    h0, hn = (3 * g, 3) if g < 42 else (126, 2)
                    lhs = t1s[v][:, h0:h0 + hn, :].rearrange("p l d -> p (l d)")
                    nc.tensor.matmul(out=ps[0:hn * DH, k, :],
                                     lhsT=lhs, rhs=bh[:, :])
                n_full = sum(1 for g in gs if g < 42)
                if n_full:
                    copy_rr(t2[v][0:120, gs[0]:gs[0] + n_full, :],
                            ps[0:120, 0:n_full, :])
                if gs[-1] == 42:
                    copy_rr(t2[v][0:80, 42, :], ps[0:80, len(gs) - 1, :])

            def ptw_chunk(ci, g0, ng, P, F, Kk, lhs3):
                ps5 = []
                for v in range(5):
                    pt = ps3p.tile([96, 512], F32, tag="ps3")
                    nc.tensor.matmul(
                        out=pt[0:P, 0:F],
                        lhsT=lhs3,
                        rhs=t2[v][0:Kk, g0:g0 + ng, :].rearrange(
                            "p g w -> p (g w)"))
                    ps5.append(pt)
                sA = ps5[0][0:P, 0:F]
                sB = ps5[1][0:P, 0:F]
                sC = ps5[2][0:P, 0:F]
                sD = ps5[3][0:P, 0:F]
                sE = ps5[4][0:P, 0:F]

                def st(tag, dt=FP16):
                    return ptw.tile([96, 512], dt, tag=tag,
                                    name=tag)[0:P, 0:F]

                qA, qB, bB, bD = st("qA"), st("qB"), st("bB"), st("bD")
                nc.scalar.activation(qA, sA, ACTF.Square)
                nc.scalar.activation(qB, sB, ACTF.Square)
                nc.scalar.copy(bB, sB)
                nc.scalar.copy(bD, sD)

                Pm, cross, Iv = st("Pm"), st("cross"), st("Iv")
                nc.vector.scalar_tensor_tensor(out=Pm, in0=sA, scalar=1.0,
                                               in1=bB, op0=ALU.bypass,
                                               op1=ALU.mult)
                nc.vector.scalar_tensor_tensor(out=cross, in0=sE, scalar=1.0,
                                               in1=Pm, op0=ALU.bypass,
                                               op1=ALU.subtract)
                nc.vector.scalar_tensor_tensor(out=Iv, in0=sC, scalar=1.0,
                                               in1=qA, op0=ALU.bypass,
                                               op1=ALU.subtract)
                Jv, num, dene = st("Jv"), st("num"), st("dene", F32)
                nc.gpsimd.scalar_tensor_tensor(out=Jv, in0=bD, scalar=1.0,
                                               in1=qB, op0=ALU.bypass,
                                               op1=ALU.subtract)
                nc.gpsimd.scalar_tensor_tensor(out=num, in0=cross, scalar=1.0,
                                               in1=cross, op0=ALU.bypass,
                                               op1=ALU.mult)
                nc.gpsimd.scalar_tensor_tensor(out=dene, in0=Iv, scalar=1.0,
                                               in1=Jv, op0=ALU.bypass,
                                               op1=ALU.mult)
                rec = st("rec", F32)
                nc.vector.reciprocal_approx_fast(out=rec, in_=dene)
                ccs = st("ccs")
                nc.vector.scalar_tensor_tensor(
                    out=ccs, in0=num, scalar=1.0, in1=rec,
                    op0=ALU.bypass, op1=ALU.mult,
                    accum_out=acc_all[0:P, ci:ci + 1])

            for ci in range(10):
                gs = list(range(ci * 4, ci * 4 + 4))
                for v in range(5):
                    pass2_block(v, gs)
                ptw_chunk(ci, ci * 4, 4, 96, 512, 120, b3f[0:120, 0:96])
            # groups 40..42 then chunks 10, 11
            for v in range(5):
                pass2_block(v, [40, 41, 42])
            ptw_chunk(10, 40, 2, 96, 256, 120, b3f[0:120, 0:96])
            ptw_chunk(11, 42, 1, 64, 128, 80, b3f[0:80, 0:64])

            cm_ptw.__exit__(None, None, None)
            cm_ps3.__exit__(None, None, None)
            cm_ps2.__exit__(None, None, None)
            cm_t1.__exit__(None, None, None)

            accs = accp.tile([128, 1], F32)
            nc.vector.tensor_reduce(out=accs[:, :], in_=acc_all[:, :],
                                    axis=mybir.AxisListType.X, op=ALU.add)
            nc.sync.dma_start(out=out_dram[:, :], in_=accs[:, :])

    nc.compile()
    return nc


def kernel(y_pred: np.ndarray, y_true: np.ndarray) -> np.ndarray:
    y_pred = np.ascontiguousarray(np.asarray(y_pred, dtype=np.float32))
    y_true = np.ascontiguousarray(np.asarray(y_true, dtype=np.float32))

    if "nc" not in _CACHE:
        _CACHE["nc"] = _build()
    nc = _CACHE["nc"]

    in_maps = []
    for core in range(8):
        b = core // 4
        d0 = (core % 4) * DL
        islab = np.zeros((DH, H, W), np.float32)
        jslab = np.zeros((DH, H, W), np.float32)
        lo, hi = d0 - PAD, d0 + DL + PAD
        slo, shi = max(lo, 0), min(hi, Dv)
        islab[slo - lo:shi - lo] = y_true[b, 0, slo:shi]
        jslab[slo - lo:shi - lo] = y_pred[b, 0, slo:shi]
        in_maps.append({"i_slab": islab, "j_slab": jslab})

    res = run_bass_kernel_spmd(nc, in_maps, core_ids=list(range(8)))
    total = 0.0
    for r in res.results:
        total += float(np.asarray(r["partials"], np.float64).sum())
    return np.float32(-total / N_TOT)


if __name__ == "__main__":
    rng = np.random.default_rng(0)
    yp = rng.standard_normal((B, 1, Dv, H, W), dtype=np.float32)
    yt = rng.standard_normal((B, 1, Dv, H, W), dtype=np.float32)
    print("loss:", kernel(yp, yt))
